# revision 7
# baseline (speedup 1.0000x reference)
"""Competitive binding equilibrium solver on 8 TRN2 NeuronCores.

  AF = AT / (1 + K @ BF);  BF = BT / (1 + K^T @ AF)   (100 fixed-point iters)
  C  = K * AF[:, None] * BF[None, :]

Strategy: shard K row-wise (512 rows/core). Keep the local K shard SBUF-resident
in BOTH layouts (K: [i-part, j-free] and K^T: [j-part, i-free]) in bf16, so each
of the 200 matvec passes streams from SBUF instead of HBM. Both matvecs are run
as "form B" matmuls (stationary = K tile [128,128], moving = vector [128,1]) so
the result vectors land in PSUM in partition-major layout, which feeds the next
pass / the DMA to DRAM directly. The K^T@AF partial is all-gathered across the
8 cores each iteration (16 KiB) and reduced locally on the Vector engine.
"""

import sys

if "/opt/trn_rl_repo" not in sys.path:
    sys.path.insert(0, "/opt/trn_rl_repo")

import numpy as np

import concourse.bass as bass
import concourse.mybir as mybir
import concourse.tile as tile
from concourse import bacc
from concourse import bass_utils
from concourse.bass import ds, ts
from concourse.masks import make_identity

F32 = mybir.dt.float32
BF16 = mybir.dt.bfloat16
ADD = mybir.AluOpType.add
MULT = mybir.AluOpType.mult
BYPASS = mybir.AluOpType.bypass

NA, NB = 4096, 4096
NCORES = 8
R = NA // NCORES          # 512 local rows per core
RT = R // 128             # 4 local row tiles (it)
JT = NB // 128            # 32 j tiles (jc / jt)
N_ITERS = 100


def build_program(n_iters: int = N_ITERS):
    nc = bacc.Bacc(
        "TRN2",
        target_bir_lowering=False,
        debug=False,
        num_devices=NCORES,
    )

    K_d = nc.dram_tensor("K", [R, NB], F32, kind="ExternalInput").ap()
    AT_d = nc.dram_tensor("AT", [R], F32, kind="ExternalInput").ap()
    BT_d = nc.dram_tensor("BT", [NB], F32, kind="ExternalInput").ap()
    C_d = nc.dram_tensor("C", [R, NB], F32, kind="ExternalOutput").ap()

    with tile.TileContext(nc) as tc:
        _body(tc, nc, K_d, AT_d, BT_d, C_d, n_iters)

    nc.compile()
    return nc


def _body(tc, nc, K_d, AT_d, BT_d, C_d, n_iters):
    rg = [list(range(NCORES))]

    def P(pool, shape, dtype, tag, **kw):
        return pool.tile(shape, dtype, name=tag, tag=tag, **kw)

    from contextlib import ExitStack

    es = ExitStack()
    persist = es.enter_context(tc.tile_pool(name="persist", bufs=1))
    psum_pool = es.enter_context(tc.tile_pool(name="psum", bufs=1, space="PSUM"))
    dram_pool = es.enter_context(tc.tile_pool(name="dram", bufs=1, space="DRAM"))

    # ---- persistent SBUF tensors -------------------------------------------
    k_sb = P(persist, [128, RT, NB], BF16, "k_sb")        # [i-part, it, j]
    kt_sb = P(persist, [128, JT, R], BF16, "kt_sb")       # [j-part, jc, i]
    at_sb = P(persist, [128, RT], F32, "at_sb")           # AT[it*128+p]
    bt_sb = P(persist, [128, JT], F32, "bt_sb")           # BT[jc*128+p]
    af_f = P(persist, [128, RT], F32, "af_f")
    af_bf = P(persist, [128, RT], BF16, "af_bf")
    bf_f = P(persist, [128, JT], F32, "bf_f")
    bf_bf = P(persist, [128, JT], BF16, "bf_bf")
    zsum = P(persist, [128, JT], F32, "zsum")
    t_jt = P(persist, [128, JT], F32, "t_jt")             # scratch [128,32]
    t_rt = P(persist, [128, RT], F32, "t_rt")             # scratch [128,4]
    zg_sb = P(persist, [128, NCORES, JT], F32, "zg_sb")   # gathered partials
    ident_bf = P(persist, [128, 128], BF16, "ident_bf")
    ident_f32 = P(persist, [128, 128], F32, "ident_f32")
    atbt_row = P(persist, [JT, 128], F32, "atbt_row")     # staging for transposes
    bf_row = P(persist, [JT, 128], F32, "bf_row")
    bf_flat = P(persist, [1, NB], F32, "bf_flat")
    bf_bc = P(persist, [128, NB], F32, "bf_bc")

    # ---- PSUM tensors -------------------------------------------------------
    y_ps = P(psum_pool, [128, RT], F32, "y_ps")
    z_ps = P(psum_pool, [128, JT], F32, "z_ps")
    tr_ps = P(psum_pool, [128, 128], F32, "tr_ps")
    tr_ps_bf = P(psum_pool, [128, 128], BF16, "tr_ps_bf")

    # ---- DRAM bounce buffers for the collective -----------------------------
    # Shared-DRAM collective outputs must each have a single writing
    # instruction, so allocate one pair per iteration.
    zins = [P(dram_pool, [128, JT], F32, f"zin{i}") for i in range(n_iters)]
    zgathers = [
        P(dram_pool, [128 * NCORES, JT], F32, f"zgather{i}", addr_space="Shared")
        for i in range(n_iters)
    ]
    bf_dram = P(dram_pool, [JT, 128], F32, "bf_dram")

    # ---- setup: identities --------------------------------------------------
    make_identity(nc, ident_bf[:])
    make_identity(nc, ident_f32[:])

    # ---- setup: AT [512] -> at_sb [128, 4]  (p, it) = AT[it*128+p] ----------
    nc.sync.dma_start(atbt_row[0:RT, :], AT_d.rearrange("(t p) -> t p", t=RT))
    nc.tensor.transpose(tr_ps[0:128, 0:RT], atbt_row[0:RT, :], ident_f32[0:RT, 0:RT])
    nc.vector.tensor_copy(at_sb[:], tr_ps[0:128, 0:RT])

    # ---- setup: BT [4096] -> bt_sb [128, 32]  (p, jc) = BT[jc*128+p] --------
    nc.sync.dma_start(atbt_row[:, :], BT_d.rearrange("(t p) -> t p", t=JT))
    nc.tensor.transpose(tr_ps[0:128, 0:JT], atbt_row[:, :], ident_f32[0:JT, 0:JT])
    nc.vector.tensor_copy(bt_sb[:], tr_ps[0:128, 0:JT])

    # ---- initial BF = BT ----------------------------------------------------
    nc.vector.tensor_copy(bf_f[:], bt_sb[:])
    nc.vector.tensor_copy(bf_bf[:], bt_sb[:])

    # ---- setup: K -> k_sb (bf16 cast), then PE-transpose into kt_sb ---------
    with tc.tile_pool(name="stage", bufs=2) as stage_pool:
        for it in range(RT):
            stg = stage_pool.tile([128, NB], F32, tag="stage")
            nc.sync.dma_start(stg[:], K_d[ts(it, 128), :])
            nc.vector.tensor_copy(k_sb[:, it, :], stg[:])
        for it in range(RT):
            for jc in range(JT):
                nc.tensor.transpose(
                    tr_ps_bf[:, :],
                    k_sb[:, it, ds(jc * 128, 128)],
                    ident_bf[:, :],
                )
                nc.vector.tensor_copy(kt_sb[:, jc, ts(it, 128)], tr_ps_bf[:, :])

        # ---- main fixed-point loop (fully unrolled; collectives cannot be in
        # control flow) -------------------------------------------------------
        for i in range(n_iters):
            # pass Y: y = K @ BF   (form B: lhsT = K^T tile, rhs = BF column)
            for jc in range(JT):
                for it in range(RT):
                    nc.tensor.matmul(
                        y_ps[:, ds(it, 1)],
                        kt_sb[:, jc, ts(it, 128)],
                        bf_bf[:, ds(jc, 1)],
                        start=(jc == 0 and it == 0),
                        stop=(jc == JT - 1 and it == RT - 1),
                    )
            # AF = AT / (1 + y)
            nc.vector.tensor_scalar_add(t_rt[:], y_ps[:], 1.0)
            nc.vector.reciprocal(t_rt[:], t_rt[:])
            nc.vector.tensor_tensor(af_f[:], t_rt[:], at_sb[:], MULT)
            nc.vector.tensor_copy(af_bf[:], af_f[:])

            # pass Z: z_part = K^T @ AF  (form B: lhsT = K tile, rhs = AF col)
            for it in range(RT):
                for jt in range(JT):
                    nc.tensor.matmul(
                        z_ps[:, ds(jt, 1)],
                        k_sb[:, it, ds(jt * 128, 128)],
                        af_bf[:, ds(it, 1)],
                        start=(it == 0 and jt == 0),
                        stop=(it == RT - 1 and jt == JT - 1),
                    )
            # local partial -> DRAM -> AllGather -> reduce over 8 slabs
            nc.vector.tensor_copy(t_jt[:], z_ps[:])
            nc.sync.dma_start(zins[i][:], t_jt[:])
            nc.gpsimd.collective_compute(
                "AllGather",
                BYPASS,
                replica_groups=rg,
                ins=[zins[i][:].opt()],
                outs=[zgathers[i][:].opt()],
            )
            nc.sync.dma_start(
                zg_sb[:], zgathers[i][:].rearrange("(s p) c -> p s c", s=NCORES)
            )
            nc.vector.tensor_tensor(
                zg_sb[:, 0:4, :], zg_sb[:, 0:4, :], zg_sb[:, 4:8, :], ADD
            )
            nc.vector.tensor_tensor(
                zg_sb[:, 0:2, :], zg_sb[:, 0:2, :], zg_sb[:, 2:4, :], ADD
            )
            nc.vector.tensor_tensor(zsum[:], zg_sb[:, 0, :], zg_sb[:, 1, :], ADD)

            # BF = BT / (1 + z)
            nc.vector.tensor_scalar_add(t_jt[:], zsum[:], 1.0)
            nc.vector.reciprocal(t_jt[:], t_jt[:])
            nc.vector.tensor_tensor(bf_f[:], t_jt[:], bt_sb[:], MULT)
            nc.vector.tensor_copy(bf_bf[:], bf_f[:])

        # ---- final: C = K * AF[:,None] * BF[None,:] -------------------------
        # BF (partition layout) -> flat row -> broadcast across partitions
        nc.tensor.transpose(tr_ps[0:JT, :], bf_f[:], ident_f32[:, :])
        nc.vector.tensor_copy(bf_row[:], tr_ps[0:JT, :])
        nc.sync.dma_start(bf_dram[:], bf_row[:])
        nc.sync.dma_start(bf_flat[:], bf_dram[:].rearrange("t p -> (t p)").unsqueeze(0))
        nc.gpsimd.partition_broadcast(bf_bc[:], bf_flat[:])

        for it in range(RT):
            stg = stage_pool.tile([128, NB], F32, tag="stage")
            nc.sync.dma_start(stg[:], K_d[ts(it, 128), :])
            cst = stage_pool.tile([128, NB], F32, tag="cstage")
            nc.vector.scalar_tensor_tensor(
                cst[:], stg[:], af_f[:, ds(it, 1)], bf_bc[:], MULT, MULT
            )
            nc.sync.dma_start(C_d[ts(it, 128), :], cst[:])

    es.close()


_CACHE = {}


def _get_program(n_iters: int = N_ITERS):
    if n_iters not in _CACHE:
        _CACHE[n_iters] = build_program(n_iters)
    return _CACHE[n_iters]


def kernel(AT, BT, K, n_iters: int = N_ITERS, trace: bool = False):
    nc = _get_program(n_iters)
    AT = np.ascontiguousarray(AT, dtype=np.float32)
    BT = np.ascontiguousarray(BT, dtype=np.float32)
    K = np.ascontiguousarray(K, dtype=np.float32)
    in_maps = [
        {"K": K[c * R : (c + 1) * R], "AT": AT[c * R : (c + 1) * R], "BT": BT}
        for c in range(NCORES)
    ]
    res = bass_utils.run_bass_kernel_spmd(
        nc, in_maps, core_ids=list(range(NCORES)), trace=trace
    )
    C = np.concatenate([res.results[c]["C"] for c in range(NCORES)], axis=0)
    if trace:
        kernel.last_results = res
    return C
